# revision 6
# baseline (speedup 1.0000x reference)
"""Trainium2 Bass kernel for the DeformationGraph problem.

Math: the reference computes, per batch b and vertex v,
    out[b,v,k] = sum_c W[v,c] * ( sum_d (X[b,v,d]-center[b,c,d]) * R[b,c,k,d]
                                  + center[b,c,k] + V_nodes[b,c,k] )
which factors into a vertex-independent per-node affine map:
    t[b,c,k]   = center[b,c,k] + V_nodes[b,c,k] - sum_d center[b,c,d]*R[b,c,k,d]
    out[b,v,k] = sum_d X[b,v,d] * (W @ R[..,k,d])[v]  +  (W @ t[..,k])[v]
i.e. one (V,C)@(C,48) matmul Y = W @ G (48 columns = 4 batches x 3 rows x
[3 rotation cols + translation]), then a tiny per-vertex contraction of Y
with [X,1].  The big tensors (W: 32MB, X, out) are sharded over the vertex
dimension across the 8 cores; G (C x 48) is replicated.

Device kernel per core (vertex shard Vs=6250 padded to 6272):
  for each vertex chunk (free axis):
    DMA   a  (128,n) = W^T rows   0..127   |  w1 (32,n) = W^T rows 128..159
    DMA   xd (48,n)  = [X,1] replicated rows (d-major, k-duplicated)
    PE    y (48,n) PSUM  = g0^T.T @ a  +  g1^T.T @ w1        (K=C split 128+32)
    DVE   p (48,n) SBUF  = y * xd
    PE    o (12,n) PSUM  = S.T @ p     (S = constant 0/1 summing the 4 d-slices)
    DMA   outT chunk = o
"""

import numpy as np

import concourse.mybir as mybir
import concourse.tile as tile
from concourse import bacc
from concourse.bass_utils import run_bass_kernel_spmd

B, V, C = 4, 50000, 160
N_CORES = 8
VS = V // N_CORES            # 6250 vertices per core
VSP = 6272                   # padded: 6*1024 + 128
CHUNKS = [1024] * 6 + [128]
F32 = mybir.dt.float32

# column order of G / partition order of Y,xd: j = d*12 + k*4 + b
# (d in 0..3 where d==3 is the translation/ones slot; k row; b batch)


def _build_bass():
    nc = bacc.Bacc()

    wt = nc.dram_tensor("wt", [160, VSP], F32, kind="ExternalInput")
    xd = nc.dram_tensor("xd", [48, VSP], F32, kind="ExternalInput")
    g0 = nc.dram_tensor("g0", [128, 48], F32, kind="ExternalInput")
    g1 = nc.dram_tensor("g1", [32, 48], F32, kind="ExternalInput")
    s = nc.dram_tensor("s", [48, 12], F32, kind="ExternalInput")
    outT = nc.dram_tensor("outT", [12, VSP], F32, kind="ExternalOutput")

    with tile.TileContext(nc) as tc:
        with (
            tc.tile_pool(name="gpool", bufs=1) as gpool,
            tc.tile_pool(name="apool", bufs=3) as apool,
            tc.tile_pool(name="wpool", bufs=3) as wpool,
            tc.tile_pool(name="xpool", bufs=3) as xpool,
            tc.tile_pool(name="ppool", bufs=3) as ppool,
            tc.tile_pool(name="rpool", bufs=3) as rpool,
            tc.tile_pool(name="ypool", bufs=2, space="PSUM") as ypool,
            tc.tile_pool(name="opool", bufs=2, space="PSUM") as opool,
        ):
            g0t = gpool.tile([128, 48], F32)
            nc.sync.dma_start(out=g0t[:], in_=g0[:])
            g1t = gpool.tile([32, 48], F32)
            nc.sync.dma_start(out=g1t[:], in_=g1[:])
            st = gpool.tile([48, 12], F32)
            nc.sync.dma_start(out=st[:], in_=s[:])

            c0 = 0
            for n in CHUNKS:
                sl = slice(c0, c0 + n)
                a = apool.tile([128, n], F32, tag="a")
                nc.sync.dma_start(out=a[:], in_=wt[0:128, sl])
                w1 = wpool.tile([32, n], F32, tag="w1")
                nc.sync.dma_start(out=w1[:], in_=wt[128:160, sl])
                x = xpool.tile([48, n], F32, tag="x")
                nc.sync.dma_start(out=x[:], in_=xd[:, sl])

                y = ypool.tile([48, n], F32, tag="y")
                for h in range(0, n, 512):
                    hs = slice(h, min(h + 512, n))
                    nc.tensor.matmul(y[:, hs], g0t[:], a[:, hs],
                                     start=True, stop=False)
                    nc.tensor.matmul(y[:, hs], g1t[:], w1[:, hs],
                                     start=False, stop=True)

                p = ppool.tile([48, n], F32, tag="p")
                nc.vector.tensor_mul(p[:], y[:], x[:])

                o = opool.tile([12, n], F32, tag="o")
                for h in range(0, n, 512):
                    hs = slice(h, min(h + 512, n))
                    nc.tensor.matmul(o[:, hs], st[:], p[:, hs],
                                     start=True, stop=True)
                r = rpool.tile([12, n], F32, tag="r")
                nc.scalar.copy(out=r[:], in_=o[:])
                nc.sync.dma_start(out=outT[:, sl], in_=r[:])
                c0 += n
    nc.finalize()
    return nc


_NC_CACHE = None


def _get_nc():
    global _NC_CACHE
    if _NC_CACHE is None:
        _NC_CACHE = _build_bass()
    return _NC_CACHE


def _host_prep(X, V_nodes, rot6d_nodes, W_nodes, idx_nn_to_nodes):
    """Small per-node math (B*C=640 rows) + shard/layout of the big tensors."""
    X = np.asarray(X, np.float32)
    Vn = np.asarray(V_nodes, np.float32)
    d6 = np.asarray(rot6d_nodes, np.float32)
    W = np.asarray(W_nodes, np.float32)
    idx = np.asarray(idx_nn_to_nodes).astype(np.int64)

    a1, a2 = d6[..., :3], d6[..., 3:]
    eps = np.float32(1e-8)
    n1 = np.sqrt(np.sum(a1 * a1, -1, keepdims=True, dtype=np.float32))
    b1 = a1 / np.maximum(n1, eps)
    dot = np.sum(b1 * a2, -1, keepdims=True, dtype=np.float32)
    a2p = a2 - dot * b1
    n2 = np.sqrt(np.sum(a2p * a2p, -1, keepdims=True, dtype=np.float32))
    b2 = a2p / np.maximum(n2, eps)
    b3 = np.cross(b1, b2)
    R = np.stack([b1, b2, b3], axis=-2).astype(np.float32)  # (B,C,3,3) [b,c,k,d]

    center = X[:, idx, :]                                   # (B,C,3)
    t = (center + Vn - np.einsum('bcd,bckd->bck', center, R)).astype(np.float32)

    G = np.empty((C, 48), np.float32)
    for d in range(4):
        for k in range(3):
            for b in range(B):
                j = d * 12 + k * 4 + b
                G[:, j] = R[b, :, k, d] if d < 3 else t[b, :, k]

    S = np.zeros((48, 12), np.float32)
    for d in range(4):
        for r in range(12):
            S[d * 12 + r, r] = 1.0

    g0 = np.ascontiguousarray(G[0:128])
    g1 = np.ascontiguousarray(G[128:160])

    in_maps = []
    for i in range(N_CORES):
        vsl = slice(i * VS, (i + 1) * VS)
        wt = np.zeros((160, VSP), np.float32)
        wt[:, :VS] = W[vsl].T
        xd = np.zeros((48, VSP), np.float32)
        for d in range(4):
            for k in range(3):
                for b in range(B):
                    j = d * 12 + k * 4 + b
                    xd[j, :VS] = X[b, vsl, d] if d < 3 else 1.0
        in_maps.append({"wt": wt, "xd": xd, "g0": g0, "g1": g1, "s": S})
    return in_maps


def _gather(results):
    out = np.empty((B, V, 3), np.float32)
    for i, res in enumerate(results):
        oT = res["outT"]
        vsl = slice(i * VS, (i + 1) * VS)
        for k in range(3):
            for b in range(B):
                out[b, vsl, k] = oT[k * 4 + b, :VS]
    return out


def kernel(X, V_nodes, rot6d_nodes, W_nodes, idx_nn_to_nodes, **run_kwargs):
    in_maps = _host_prep(X, V_nodes, rot6d_nodes, W_nodes, idx_nn_to_nodes)
    res = run_bass_kernel_spmd(_get_nc(), in_maps,
                               core_ids=list(range(N_CORES)), **run_kwargs)
    out = _gather(res.results)
    kernel.last_run = res
    return out


# revision 11
# speedup vs baseline: 1.1698x; 1.1698x over previous
"""Trainium2 Bass kernel for the DeformationGraph problem.

Math: the reference computes, per batch b and vertex v,
    out[b,v,k] = sum_c W[v,c] * ( sum_d (X[b,v,d]-center[b,c,d]) * R[b,c,k,d]
                                  + center[b,c,k] + V_nodes[b,c,k] )
which factors into a vertex-independent per-node affine map:
    t[b,c,k]   = center[b,c,k] + V_nodes[b,c,k] - sum_d center[b,c,d]*R[b,c,k,d]
    out[b,v,k] = sum_d X[b,v,d] * (W @ R[..,k,d])[v]  +  (W @ t[..,k])[v]
i.e. one (V,C)@(C,48) matmul Y = W @ G, then a tiny per-vertex contraction
of Y with [X,1].  The big tensors (W: 32MB, X, out) are sharded over the
vertex dimension across the 8 cores; G (C x 48) is replicated.

Layouts: the 48 Y rows live at PSUM/SBUF partitions j = d*32 + (k*4 + b)
(d in 0..3 with d==3 the translation/ones slot; unused cols of each 32-block
are zero) so the d-reduction's operand slices are 32-aligned, which the
engines require.

fp32 matmul on TRN2 runs in LOW_HIGH dual-pass mode (~5x slower), so the
matmul uses the exact-enough 3-term bf16 split:
    W @ G ~= Wh@Gh + Wl@Gh + Wh@Gl     (Wh=bf16(W), Wl=bf16(W-Wh), ...)
measured end-to-end error vs the fp32 reference: ~4e-6 absmax.

The contraction dim C=160 splits into an A part (c 0..127, K=128) and a B
part (c 128..159, K=32).  The three B-part terms are packed into one K=96
matmul by stacking [WhB; WhB; WlB] against [GhB; GlB; GhB] host-side.

Device kernel per core (vertex shard Vs=6250 padded to 6272):
  for each vertex chunk n (free axis):
    DMA   wha(128,n) wla(128,n) bpk(96,n) bf16;  xdt(128,n) f32 (4 block DMAs)
    PE    y(128,n) PSUM f32 += gh0.T@wha + gl0.T@wha + gh0.T@wla + gbk.T@bpk
          (per 512-wide half; 8 matmuls per 1024 chunk)
    DVE   p (128,n) SBUF = y * xdt
    ACT   q (64,n) = copy p[64:128]    (engines need equal SBUF base partitions
    POOL  a64 (64,n) = p[0:64] + q      on 2-input ops, so shift-copy first)
    ACT   r32 (32,n) = copy a64[32:64]
    DVE   o (12,n) = a64[0:12] + r32[0:12]
    DMA   outT chunk = o
"""

import numpy as np
import ml_dtypes

import concourse.mybir as mybir
import concourse.tile as tile
from concourse import bacc
from concourse.bass_utils import run_bass_kernel_spmd

B, V, C = 4, 50000, 160
N_CORES = 8
VS = V // N_CORES            # 6250 vertices per core
VSP = 6272                   # padded: 6*1024 + 128
CHUNKS = [1024] * 6 + [128]
F32 = mybir.dt.float32
BF16 = mybir.dt.bfloat16
NPBF16 = ml_dtypes.bfloat16


def _build_bass():
    nc = bacc.Bacc()

    wha_d = nc.dram_tensor("wha", [128, VSP], BF16, kind="ExternalInput")
    wla_d = nc.dram_tensor("wla", [128, VSP], BF16, kind="ExternalInput")
    wb_d = nc.dram_tensor("wb", [96, VSP], BF16, kind="ExternalInput")
    xd_d = nc.dram_tensor("xd", [48, VSP], F32, kind="ExternalInput")
    gh0_d = nc.dram_tensor("gh0", [128, 128], BF16, kind="ExternalInput")
    gl0_d = nc.dram_tensor("gl0", [128, 128], BF16, kind="ExternalInput")
    gbk_d = nc.dram_tensor("gbk", [96, 128], BF16, kind="ExternalInput")
    outT = nc.dram_tensor("outT", [12, VSP], F32, kind="ExternalOutput")

    with tile.TileContext(nc) as tc:
        with (
            tc.tile_pool(name="gpool", bufs=1) as gpool,
            tc.tile_pool(name="wpool", bufs=3) as wpool,
            tc.tile_pool(name="xpool", bufs=3) as xpool,
            tc.tile_pool(name="ppool", bufs=3) as ppool,
            tc.tile_pool(name="qpool", bufs=3) as qpool,
            tc.tile_pool(name="apool", bufs=3) as apool,
            tc.tile_pool(name="rpool", bufs=3) as rpool,
            tc.tile_pool(name="opool", bufs=3) as opool,
            tc.tile_pool(name="ypool", bufs=3, space="PSUM") as ypool,
        ):
            gh0 = gpool.tile([128, 128], BF16)
            nc.sync.dma_start(out=gh0[:], in_=gh0_d[:])
            gl0 = gpool.tile([128, 128], BF16)
            nc.sync.dma_start(out=gl0[:], in_=gl0_d[:])
            gbk = gpool.tile([96, 128], BF16)
            nc.sync.dma_start(out=gbk[:], in_=gbk_d[:])

            c0 = 0
            for n in CHUNKS:
                sl = slice(c0, c0 + n)
                wha = wpool.tile([128, n], BF16, tag="wha")
                nc.sync.dma_start(out=wha[:], in_=wha_d[:, sl])
                wla = wpool.tile([128, n], BF16, tag="wla")
                nc.sync.dma_start(out=wla[:], in_=wla_d[:, sl])
                bpk = wpool.tile([96, n], BF16, tag="bpk")
                nc.sync.dma_start(out=bpk[:], in_=wb_d[:, sl])
                xdt = xpool.tile([128, n], F32, tag="xdt")
                for s in range(4):
                    nc.sync.dma_start(out=xdt[s * 32:s * 32 + 12, :],
                                      in_=xd_d[s * 12:(s + 1) * 12, sl])

                y = ypool.tile([128, n], F32, tag="y")
                for h in range(0, n, 512):
                    hs = slice(h, min(h + 512, n))
                    nc.tensor.matmul(y[:, hs], gh0[:], wha[:, hs],
                                     start=True, stop=False)
                    nc.tensor.matmul(y[:, hs], gl0[:], wha[:, hs],
                                     start=False, stop=False)
                    nc.tensor.matmul(y[:, hs], gh0[:], wla[:, hs],
                                     start=False, stop=False)
                    nc.tensor.matmul(y[:, hs], gbk[:], bpk[:, hs],
                                     start=False, stop=True)

                p = ppool.tile([128, n], F32, tag="p")
                nc.vector.tensor_mul(out=p[:], in0=y[:], in1=xdt[:])
                q = qpool.tile([64, n], F32, tag="q")
                nc.scalar.copy(out=q[:], in_=p[64:128, :])
                a64 = apool.tile([64, n], F32, tag="a64")
                nc.gpsimd.tensor_add(out=a64[:], in0=p[0:64, :], in1=q[:])
                r32 = rpool.tile([32, n], F32, tag="r32")
                nc.scalar.copy(out=r32[:], in_=a64[32:64, :])
                o = opool.tile([12, n], F32, tag="o")
                nc.vector.tensor_add(out=o[:], in0=a64[0:12, :], in1=r32[0:12, :])
                nc.sync.dma_start(out=outT[:, sl], in_=o[:])
                c0 += n
    nc.finalize()
    return nc


_NC_CACHE = None


def _get_nc():
    global _NC_CACHE
    if _NC_CACHE is None:
        _NC_CACHE = _build_bass()
    return _NC_CACHE


def _host_prep(X, V_nodes, rot6d_nodes, W_nodes, idx_nn_to_nodes):
    """Small per-node math (B*C=640 rows) + shard/layout of the big tensors."""
    X = np.asarray(X, np.float32)
    Vn = np.asarray(V_nodes, np.float32)
    d6 = np.asarray(rot6d_nodes, np.float32)
    W = np.asarray(W_nodes, np.float32)
    idx = np.asarray(idx_nn_to_nodes).astype(np.int64)

    a1, a2 = d6[..., :3], d6[..., 3:]
    eps = np.float32(1e-8)
    n1 = np.sqrt(np.sum(a1 * a1, -1, keepdims=True, dtype=np.float32))
    b1 = a1 / np.maximum(n1, eps)
    dot = np.sum(b1 * a2, -1, keepdims=True, dtype=np.float32)
    a2p = a2 - dot * b1
    n2 = np.sqrt(np.sum(a2p * a2p, -1, keepdims=True, dtype=np.float32))
    b2 = a2p / np.maximum(n2, eps)
    b3 = np.cross(b1, b2)
    R = np.stack([b1, b2, b3], axis=-2).astype(np.float32)  # (B,C,3,3) [b,c,k,d]

    center = X[:, idx, :]                                   # (B,C,3)
    t = (center + Vn - np.einsum('bcd,bckd->bck', center, R)).astype(np.float32)

    # G columns at j = d*32 + k*4 + b; cols 12..31 of each block zero
    G = np.zeros((C, 128), np.float32)
    for d in range(4):
        for k in range(3):
            for b in range(B):
                j = d * 32 + k * 4 + b
                G[:, j] = R[b, :, k, d] if d < 3 else t[b, :, k]

    Gh = G.astype(NPBF16)
    Gl = (G - Gh.astype(np.float32)).astype(NPBF16)
    gh0 = np.ascontiguousarray(Gh[0:128])
    gl0 = np.ascontiguousarray(Gl[0:128])
    gbk = np.ascontiguousarray(
        np.concatenate([Gh[128:160], Gl[128:160], Gh[128:160]], axis=0))

    Wh = W.astype(NPBF16)
    Wl = (W - Wh.astype(np.float32)).astype(NPBF16)

    in_maps = []
    for i in range(N_CORES):
        vsl = slice(i * VS, (i + 1) * VS)
        wht = np.zeros((160, VSP), NPBF16)
        wht[:, :VS] = Wh[vsl].T
        wlt = np.zeros((160, VSP), NPBF16)
        wlt[:, :VS] = Wl[vsl].T
        wha = np.ascontiguousarray(wht[0:128])
        wla = np.ascontiguousarray(wlt[0:128])
        wb = np.ascontiguousarray(
            np.concatenate([wht[128:160], wht[128:160], wlt[128:160]], axis=0))
        # xd rows s*12 + r with r = k*4+b: X[b,:,d] for s=d<3, ones for s=3
        xd = np.zeros((48, VSP), np.float32)
        for d in range(4):
            for k in range(3):
                for b in range(B):
                    r = d * 12 + k * 4 + b
                    xd[r, :VS] = X[b, vsl, d] if d < 3 else 1.0
        in_maps.append({"wha": wha, "wla": wla, "wb": wb, "xd": xd,
                        "gh0": gh0, "gl0": gl0, "gbk": gbk})
    return in_maps


def _gather(results):
    out = np.empty((B, V, 3), np.float32)
    for i, res in enumerate(results):
        oT = res["outT"]
        vsl = slice(i * VS, (i + 1) * VS)
        for k in range(3):
            for b in range(B):
                out[b, vsl, k] = oT[k * 4 + b, :VS]
    return out


def kernel(X, V_nodes, rot6d_nodes, W_nodes, idx_nn_to_nodes, **run_kwargs):
    in_maps = _host_prep(X, V_nodes, rot6d_nodes, W_nodes, idx_nn_to_nodes)
    res = run_bass_kernel_spmd(_get_nc(), in_maps,
                               core_ids=list(range(N_CORES)), **run_kwargs)
    out = _gather(res.results)
    kernel.last_run = res
    return out


# revision 15
# speedup vs baseline: 1.2587x; 1.0760x over previous
"""Trainium2 Bass kernel for the DeformationGraph problem.

Math: the reference computes, per batch b and vertex v,
    out[b,v,k] = sum_c W[v,c] * ( sum_d (X[b,v,d]-center[b,c,d]) * R[b,c,k,d]
                                  + center[b,c,k] + V_nodes[b,c,k] )
which factors into a vertex-independent per-node affine map:
    t[b,c,k]   = center[b,c,k] + V_nodes[b,c,k] - sum_d center[b,c,d]*R[b,c,k,d]
    out[b,v,k] = sum_d X[b,v,d] * (W @ R[..,k,d])[v]  +  (W @ t[..,k])[v]
i.e. one (V,C)@(C,48) matmul Y = W @ G, then a tiny per-vertex contraction
of Y with [X,1].  The big tensors (W: 32MB, X, out) are sharded over the
vertex dimension across the 8 cores; G (C x 48) is replicated.

Layouts: the 48 Y rows live at PSUM/SBUF partitions j = d*32 + (k*4 + b)
(d in 0..3 with d==3 the translation/ones slot; unused rows of each
32-block are zero) so every 2-input engine op sees equal base partitions,
which the ISA requires for SBUF+SBUF operand pairs.

fp32 matmul on TRN2 runs in LOW_HIGH dual-pass mode (~5x slower), so the
matmul uses the exact-enough 3-term bf16 split:
    W @ G ~= Wh@Gh + Wl@Gh + Wh@Gl     (Wh=bf16(W), Wl=bf16(W-Wh), ...)
measured end-to-end error vs the fp32 reference: ~4e-6 absmax.

The contraction dim C=160 splits into an A part (c 0..127, K=128) and a B
part (c 128..159, K=32).  The three B-part terms are packed into one K=96
matmul by stacking [WhB; WhB; WlB] against [GhB; GlB; GhB] host-side.

Device kernel per core (vertex shard Vs=6250 padded to 6272), DMAs batched
in 3072-wide macro chunks to amortize the ~0.7us per-DMA issue cost on the
sequencer queues; compute runs in 1024-wide sub-chunks:
  per macro chunk m (3072):
    DMA(SP)   wha(128,m) wla(128,m) bpk(96,m) bf16
    DMA(ACT)  xdt(128,m) f32, one DMA into the 4 partition blocks
    per 1024 sub-chunk n:
      PE    y(128,n) PSUM f32 += gh0.T@wha + gl0.T@wha + gh0.T@wla + gbk.T@bpk
            (per 512-wide half; 8 matmuls per sub-chunk)
      DVE   p (128,n) SBUF = y * xdt
      ACT   q (64,n) = copy p[64:128]          (partition shift 64 -> 0)
      POOL  a64[0:32]  = p[0:32]  + q[0:32]    (s0 + s2)
      DVE   a64[32:64] = p[32:64] + q[32:64]   (s1 + s3)
    DMA(Q7)   os (12,m) SBUF = a64[0:12]; os += a64[32:44]  (SWDGE accum)
    DMA(SP)   outT[:, m] = os
"""

import numpy as np
import ml_dtypes

import concourse.mybir as mybir
import concourse.tile as tile
from concourse import bacc
from concourse.bass_utils import run_bass_kernel_spmd

B, V, C = 4, 50000, 160
N_CORES = 8
VS = V // N_CORES            # 6250 vertices per core
VSP = 6272                   # padded: 2*3072 + 128
MACROS = [3072, 3072, 128]
SUB = 1024
F32 = mybir.dt.float32
BF16 = mybir.dt.bfloat16
NPBF16 = ml_dtypes.bfloat16


def _build_bass():
    nc = bacc.Bacc()

    wha_d = nc.dram_tensor("wha", [128, VSP], BF16, kind="ExternalInput")
    wla_d = nc.dram_tensor("wla", [128, VSP], BF16, kind="ExternalInput")
    wb_d = nc.dram_tensor("wb", [96, VSP], BF16, kind="ExternalInput")
    xd_d = nc.dram_tensor("xd", [48, VSP], F32, kind="ExternalInput")
    gh0_d = nc.dram_tensor("gh0", [128, 128], BF16, kind="ExternalInput")
    gl0_d = nc.dram_tensor("gl0", [128, 128], BF16, kind="ExternalInput")
    gbk_d = nc.dram_tensor("gbk", [96, 128], BF16, kind="ExternalInput")
    outT = nc.dram_tensor("outT", [12, VSP], F32, kind="ExternalOutput")

    with tile.TileContext(nc) as tc:
        with (
            tc.tile_pool(name="gpool", bufs=1) as gpool,
            tc.tile_pool(name="wpool", bufs=2) as wpool,
            tc.tile_pool(name="xpool", bufs=2) as xpool,
            tc.tile_pool(name="ppool", bufs=3) as ppool,
            tc.tile_pool(name="qpool", bufs=3) as qpool,
            tc.tile_pool(name="apool", bufs=2) as apool,
            tc.tile_pool(name="ospool", bufs=2) as ospool,
            tc.tile_pool(name="ypool", bufs=3, space="PSUM") as ypool,
        ):
            gh0 = gpool.tile([128, 128], BF16)
            nc.sync.dma_start(out=gh0[:], in_=gh0_d[:])
            gl0 = gpool.tile([128, 128], BF16)
            nc.sync.dma_start(out=gl0[:], in_=gl0_d[:])
            gbk = gpool.tile([96, 128], BF16)
            nc.sync.dma_start(out=gbk[:], in_=gbk_d[:])

            m0 = 0
            for mn in MACROS:
                msl = slice(m0, m0 + mn)
                wha = wpool.tile([128, mn], BF16, tag="wha")
                nc.sync.dma_start(out=wha[:], in_=wha_d[:, msl])
                wla = wpool.tile([128, mn], BF16, tag="wla")
                nc.sync.dma_start(out=wla[:], in_=wla_d[:, msl])
                bpk = wpool.tile([96, mn], BF16, tag="bpk")
                nc.sync.dma_start(out=bpk[:], in_=wb_d[:, msl])
                xdt = xpool.tile([128, mn], F32, tag="xdt")
                for s in range(4):
                    nc.scalar.dma_start(out=xdt[s * 32:s * 32 + 12, :],
                                        in_=xd_d[s * 12:(s + 1) * 12, msl])

                a64 = apool.tile([64, mn], F32, tag="a64")

                for u0 in range(0, mn, SUB):
                    n = min(SUB, mn - u0)
                    us = slice(u0, u0 + n)
                    y = ypool.tile([128, n], F32, tag="y")
                    for h in range(0, n, 512):
                        hs = slice(u0 + h, u0 + min(h + 512, n))
                        ys = slice(h, min(h + 512, n))
                        nc.tensor.matmul(y[:, ys], gh0[:], wha[:, hs],
                                         start=True, stop=False)
                        nc.tensor.matmul(y[:, ys], gl0[:], wha[:, hs],
                                         start=False, stop=False)
                        nc.tensor.matmul(y[:, ys], gh0[:], wla[:, hs],
                                         start=False, stop=False)
                        nc.tensor.matmul(y[:, ys], gbk[:], bpk[:, hs],
                                         start=False, stop=True)

                    p = ppool.tile([128, n], F32, tag="p")
                    nc.vector.tensor_mul(out=p[:], in0=y[:], in1=xdt[:, us])
                    q = qpool.tile([64, n], F32, tag="q")
                    nc.scalar.copy(out=q[:], in_=p[64:128, :])
                    nc.gpsimd.tensor_add(out=a64[0:32, us], in0=p[0:32, :],
                                         in1=q[0:32, :])
                    nc.vector.tensor_add(out=a64[32:64, us], in0=p[32:64, :],
                                         in1=q[32:64, :])

                r32 = ospool.tile([32, mn], F32, tag="r32")
                nc.scalar.copy(out=r32[:], in_=a64[32:64, :])
                os_ = ospool.tile([12, mn], F32, tag="os")
                nc.vector.tensor_add(out=os_[:], in0=a64[0:12, :],
                                     in1=r32[0:12, :])
                nc.sync.dma_start(out=outT[:, msl], in_=os_[:])
                m0 += mn
    nc.finalize()
    return nc


_NC_CACHE = None


def _get_nc():
    global _NC_CACHE
    if _NC_CACHE is None:
        _NC_CACHE = _build_bass()
    return _NC_CACHE


def _host_prep(X, V_nodes, rot6d_nodes, W_nodes, idx_nn_to_nodes):
    """Small per-node math (B*C=640 rows) + shard/layout of the big tensors."""
    X = np.asarray(X, np.float32)
    Vn = np.asarray(V_nodes, np.float32)
    d6 = np.asarray(rot6d_nodes, np.float32)
    W = np.asarray(W_nodes, np.float32)
    idx = np.asarray(idx_nn_to_nodes).astype(np.int64)

    a1, a2 = d6[..., :3], d6[..., 3:]
    eps = np.float32(1e-8)
    n1 = np.sqrt(np.sum(a1 * a1, -1, keepdims=True, dtype=np.float32))
    b1 = a1 / np.maximum(n1, eps)
    dot = np.sum(b1 * a2, -1, keepdims=True, dtype=np.float32)
    a2p = a2 - dot * b1
    n2 = np.sqrt(np.sum(a2p * a2p, -1, keepdims=True, dtype=np.float32))
    b2 = a2p / np.maximum(n2, eps)
    b3 = np.cross(b1, b2)
    R = np.stack([b1, b2, b3], axis=-2).astype(np.float32)  # (B,C,3,3) [b,c,k,d]

    center = X[:, idx, :]                                   # (B,C,3)
    t = (center + Vn - np.einsum('bcd,bckd->bck', center, R)).astype(np.float32)

    # G columns at j = d*32 + k*4 + b; cols 12..31 of each block zero
    G = np.zeros((C, 128), np.float32)
    for d in range(4):
        for k in range(3):
            for b in range(B):
                j = d * 32 + k * 4 + b
                G[:, j] = R[b, :, k, d] if d < 3 else t[b, :, k]

    Gh = G.astype(NPBF16)
    Gl = (G - Gh.astype(np.float32)).astype(NPBF16)
    gh0 = np.ascontiguousarray(Gh[0:128])
    gl0 = np.ascontiguousarray(Gl[0:128])
    gbk = np.ascontiguousarray(
        np.concatenate([Gh[128:160], Gl[128:160], Gh[128:160]], axis=0))

    Wh = W.astype(NPBF16)
    Wl = (W - Wh.astype(np.float32)).astype(NPBF16)

    in_maps = []
    for i in range(N_CORES):
        vsl = slice(i * VS, (i + 1) * VS)
        wht = np.zeros((160, VSP), NPBF16)
        wht[:, :VS] = Wh[vsl].T
        wlt = np.zeros((160, VSP), NPBF16)
        wlt[:, :VS] = Wl[vsl].T
        wha = np.ascontiguousarray(wht[0:128])
        wla = np.ascontiguousarray(wlt[0:128])
        wb = np.ascontiguousarray(
            np.concatenate([wht[128:160], wht[128:160], wlt[128:160]], axis=0))
        # xd rows s*12 + r with r = k*4+b: X[b,:,d] for s=d<3, ones for s=3
        xd = np.zeros((48, VSP), np.float32)
        for d in range(4):
            for k in range(3):
                for b in range(B):
                    r = d * 12 + k * 4 + b
                    xd[r, :VS] = X[b, vsl, d] if d < 3 else 1.0
        in_maps.append({"wha": wha, "wla": wla, "wb": wb, "xd": xd,
                        "gh0": gh0, "gl0": gl0, "gbk": gbk})
    return in_maps


def _gather(results):
    out = np.empty((B, V, 3), np.float32)
    for i, res in enumerate(results):
        oT = res["outT"]
        vsl = slice(i * VS, (i + 1) * VS)
        for k in range(3):
            for b in range(B):
                out[b, vsl, k] = oT[k * 4 + b, :VS]
    return out


def kernel(X, V_nodes, rot6d_nodes, W_nodes, idx_nn_to_nodes, **run_kwargs):
    in_maps = _host_prep(X, V_nodes, rot6d_nodes, W_nodes, idx_nn_to_nodes)
    res = run_bass_kernel_spmd(_get_nc(), in_maps,
                               core_ids=list(range(N_CORES)), **run_kwargs)
    out = _gather(res.results)
    kernel.last_run = res
    return out


# revision 17
# speedup vs baseline: 1.4027x; 1.1144x over previous
"""Trainium2 Bass kernel for the DeformationGraph problem.

Math: the reference computes, per batch b and vertex v,
    out[b,v,k] = sum_c W[v,c] * ( sum_d (X[b,v,d]-center[b,c,d]) * R[b,c,k,d]
                                  + center[b,c,k] + V_nodes[b,c,k] )
which factors into a vertex-independent per-node affine map:
    t[b,c,k]   = center[b,c,k] + V_nodes[b,c,k] - sum_d center[b,c,d]*R[b,c,k,d]
    out[b,v,k] = sum_d X[b,v,d] * (W @ R[..,k,d])[v]  +  (W @ t[..,k])[v]
i.e. one (V,C)@(C,48) matmul Y = W @ G, then a tiny per-vertex contraction
of Y with [X,1].  The big tensors (W: 32MB, X, out) are sharded over the
vertex dimension across the 8 cores; G (C x 48) is replicated.

Layouts: the 48 Y rows live at PSUM/SBUF partitions j = d*32 + (k*4 + b)
(d in 0..3 with d==3 the translation/ones slot; unused rows of each
32-block are zero) so every 2-input engine op sees equal base partitions,
which the ISA requires for SBUF+SBUF operand pairs.

fp32 matmul on TRN2 runs in LOW_HIGH dual-pass mode (~5x slower), so the
matmul uses the exact-enough 3-term bf16 split:
    W @ G ~= Wh@Gh + Wl@Gh + Wh@Gl     (Wh=bf16(W), Wl=bf16(W-Wh), ...)
measured end-to-end error vs the fp32 reference: ~4e-6 absmax.

The contraction dim C=160 splits into an A part (c 0..127, K=128) and a B
part (c 128..159, K=32).  The three B-part terms are packed into one K=96
matmul by stacking [WhB; WhB; WlB] against [GhB; GlB; GhB] host-side.

Device kernel per core (vertex shard Vs=6250 padded to 6272), DMAs batched
in 3072-wide macro chunks to amortize the ~0.7us per-DMA issue cost on the
sequencer queues; compute runs in 1024-wide sub-chunks:
  per macro chunk m (3072):
    DMA(SP)   wha(128,m) wla(128,m) bpk(96,m) bf16
    DMA(ACT)  xdt(128,m) f32, one DMA into the 4 partition blocks
    per 1024 sub-chunk n:
      PE    y(128,n) PSUM f32 += gh0.T@wha + gl0.T@wha + gh0.T@wla + gbk.T@bpk
            (per 512-wide half; 8 matmuls per sub-chunk)
      DVE   p (128,n) SBUF = y * xdt
      ACT   q (64,n) = copy p[64:128]          (partition shift 64 -> 0)
      POOL  a64[0:32]  = p[0:32]  + q[0:32]    (s0 + s2)
      DVE   a64[32:64] = p[32:64] + q[32:64]   (s1 + s3)
    DMA(Q7)   os (12,m) SBUF = a64[0:12]; os += a64[32:44]  (SWDGE accum)
    DMA(SP)   outT[:, m] = os
"""

import numpy as np
import ml_dtypes

import concourse.mybir as mybir
import concourse.tile as tile
from concourse import bacc
from concourse.bass_utils import run_bass_kernel_spmd

B, V, C = 4, 50000, 160
N_CORES = 8
VS = V // N_CORES            # 6250 vertices per core
VSP = 6272                   # padded: 2*3072 + 128
MACROS = [3072, 3072, 128]
SUB = 1024
F32 = mybir.dt.float32
BF16 = mybir.dt.bfloat16
NPBF16 = ml_dtypes.bfloat16


def _build_bass():
    nc = bacc.Bacc()

    wha_d = nc.dram_tensor("wha", [128, VSP], BF16, kind="ExternalInput")
    wla_d = nc.dram_tensor("wla", [128, VSP], BF16, kind="ExternalInput")
    wb_d = nc.dram_tensor("wb", [96, VSP], BF16, kind="ExternalInput")
    xd_d = nc.dram_tensor("xd", [48, VSP], F32, kind="ExternalInput")
    gh0_d = nc.dram_tensor("gh0", [128, 128], BF16, kind="ExternalInput")
    gl0_d = nc.dram_tensor("gl0", [128, 128], BF16, kind="ExternalInput")
    gbk_d = nc.dram_tensor("gbk", [96, 128], BF16, kind="ExternalInput")
    outT = nc.dram_tensor("outT", [12, VSP], F32, kind="ExternalOutput")

    with tile.TileContext(nc) as tc:
        with (
            tc.tile_pool(name="gpool", bufs=1) as gpool,
            tc.tile_pool(name="wpool", bufs=2) as wpool,
            tc.tile_pool(name="xpool", bufs=2) as xpool,
            tc.tile_pool(name="ppool", bufs=3) as ppool,
            tc.tile_pool(name="qpool", bufs=3) as qpool,
            tc.tile_pool(name="apool", bufs=3) as apool,
            tc.tile_pool(name="ospool", bufs=2) as ospool,
            tc.tile_pool(name="ypool", bufs=3, space="PSUM") as ypool,
        ):
            gh0 = gpool.tile([128, 128], BF16)
            nc.sync.dma_start(out=gh0[:], in_=gh0_d[:])
            gl0 = gpool.tile([128, 128], BF16)
            nc.sync.dma_start(out=gl0[:], in_=gl0_d[:])
            gbk = gpool.tile([96, 128], BF16)
            nc.sync.dma_start(out=gbk[:], in_=gbk_d[:])

            m0 = 0
            for mn in MACROS:
                msl = slice(m0, m0 + mn)
                wha = wpool.tile([128, mn], BF16, tag="wha")
                nc.sync.dma_start(out=wha[:], in_=wha_d[:, msl])
                wla = wpool.tile([128, mn], BF16, tag="wla")
                nc.sync.dma_start(out=wla[:], in_=wla_d[:, msl])
                bpk = wpool.tile([96, mn], BF16, tag="bpk")
                nc.sync.dma_start(out=bpk[:], in_=wb_d[:, msl])
                xdt = xpool.tile([128, mn], F32, tag="xdt")
                for s in range(4):
                    nc.gpsimd.dma_start(out=xdt[s * 32:s * 32 + 12, :],
                                        in_=xd_d[s * 12:(s + 1) * 12, msl])

                os_ = ospool.tile([12, mn], F32, tag="os")

                for u0 in range(0, mn, SUB):
                    n = min(SUB, mn - u0)
                    us = slice(u0, u0 + n)
                    y = ypool.tile([128, n], F32, tag="y")
                    for h in range(0, n, 512):
                        hs = slice(u0 + h, u0 + min(h + 512, n))
                        ys = slice(h, min(h + 512, n))
                        nc.tensor.matmul(y[:, ys], gh0[:], wha[:, hs],
                                         start=True, stop=False)
                        nc.tensor.matmul(y[:, ys], gl0[:], wha[:, hs],
                                         start=False, stop=False)
                        nc.tensor.matmul(y[:, ys], gh0[:], wla[:, hs],
                                         start=False, stop=False)
                        nc.tensor.matmul(y[:, ys], gbk[:], bpk[:, hs],
                                         start=False, stop=True)

                    p = ppool.tile([128, n], F32, tag="p")
                    nc.vector.tensor_mul(out=p[:], in0=y[:], in1=xdt[:, us])
                    q = qpool.tile([64, n], F32, tag="q")
                    nc.scalar.copy(out=q[:], in_=p[64:128, :])
                    a = apool.tile([64, n], F32, tag="a")
                    nc.vector.tensor_add(out=a[:], in0=p[0:64, :], in1=q[:])
                    r = qpool.tile([32, n], F32, tag="r")
                    nc.scalar.copy(out=r[:], in_=a[32:64, :])
                    nc.vector.tensor_add(out=os_[:, us], in0=a[0:12, :],
                                         in1=r[0:12, :])

                nc.sync.dma_start(out=outT[:, msl], in_=os_[:])
                m0 += mn
    nc.finalize()
    return nc


_NC_CACHE = None


def _get_nc():
    global _NC_CACHE
    if _NC_CACHE is None:
        _NC_CACHE = _build_bass()
    return _NC_CACHE


def _host_prep(X, V_nodes, rot6d_nodes, W_nodes, idx_nn_to_nodes):
    """Small per-node math (B*C=640 rows) + shard/layout of the big tensors."""
    X = np.asarray(X, np.float32)
    Vn = np.asarray(V_nodes, np.float32)
    d6 = np.asarray(rot6d_nodes, np.float32)
    W = np.asarray(W_nodes, np.float32)
    idx = np.asarray(idx_nn_to_nodes).astype(np.int64)

    a1, a2 = d6[..., :3], d6[..., 3:]
    eps = np.float32(1e-8)
    n1 = np.sqrt(np.sum(a1 * a1, -1, keepdims=True, dtype=np.float32))
    b1 = a1 / np.maximum(n1, eps)
    dot = np.sum(b1 * a2, -1, keepdims=True, dtype=np.float32)
    a2p = a2 - dot * b1
    n2 = np.sqrt(np.sum(a2p * a2p, -1, keepdims=True, dtype=np.float32))
    b2 = a2p / np.maximum(n2, eps)
    b3 = np.cross(b1, b2)
    R = np.stack([b1, b2, b3], axis=-2).astype(np.float32)  # (B,C,3,3) [b,c,k,d]

    center = X[:, idx, :]                                   # (B,C,3)
    t = (center + Vn - np.einsum('bcd,bckd->bck', center, R)).astype(np.float32)

    # G columns at j = d*32 + k*4 + b; cols 12..31 of each block zero
    G = np.zeros((C, 128), np.float32)
    for d in range(4):
        for k in range(3):
            for b in range(B):
                j = d * 32 + k * 4 + b
                G[:, j] = R[b, :, k, d] if d < 3 else t[b, :, k]

    Gh = G.astype(NPBF16)
    Gl = (G - Gh.astype(np.float32)).astype(NPBF16)
    gh0 = np.ascontiguousarray(Gh[0:128])
    gl0 = np.ascontiguousarray(Gl[0:128])
    gbk = np.ascontiguousarray(
        np.concatenate([Gh[128:160], Gl[128:160], Gh[128:160]], axis=0))

    Wh = W.astype(NPBF16)
    Wl = (W - Wh.astype(np.float32)).astype(NPBF16)

    in_maps = []
    for i in range(N_CORES):
        vsl = slice(i * VS, (i + 1) * VS)
        wht = np.zeros((160, VSP), NPBF16)
        wht[:, :VS] = Wh[vsl].T
        wlt = np.zeros((160, VSP), NPBF16)
        wlt[:, :VS] = Wl[vsl].T
        wha = np.ascontiguousarray(wht[0:128])
        wla = np.ascontiguousarray(wlt[0:128])
        wb = np.ascontiguousarray(
            np.concatenate([wht[128:160], wht[128:160], wlt[128:160]], axis=0))
        # xd rows s*12 + r with r = k*4+b: X[b,:,d] for s=d<3, ones for s=3
        xd = np.zeros((48, VSP), np.float32)
        for d in range(4):
            for k in range(3):
                for b in range(B):
                    r = d * 12 + k * 4 + b
                    xd[r, :VS] = X[b, vsl, d] if d < 3 else 1.0
        in_maps.append({"wha": wha, "wla": wla, "wb": wb, "xd": xd,
                        "gh0": gh0, "gl0": gl0, "gbk": gbk})
    return in_maps


def _gather(results):
    out = np.empty((B, V, 3), np.float32)
    for i, res in enumerate(results):
        oT = res["outT"]
        vsl = slice(i * VS, (i + 1) * VS)
        for k in range(3):
            for b in range(B):
                out[b, vsl, k] = oT[k * 4 + b, :VS]
    return out


def kernel(X, V_nodes, rot6d_nodes, W_nodes, idx_nn_to_nodes, **run_kwargs):
    in_maps = _host_prep(X, V_nodes, rot6d_nodes, W_nodes, idx_nn_to_nodes)
    res = run_bass_kernel_spmd(_get_nc(), in_maps,
                               core_ids=list(range(N_CORES)), **run_kwargs)
    out = _gather(res.results)
    kernel.last_run = res
    return out


# revision 21
# speedup vs baseline: 1.5914x; 1.1345x over previous
"""Trainium2 Bass kernel for the DeformationGraph problem.

Math: the reference computes, per batch b and vertex v,
    out[b,v,k] = sum_c W[v,c] * ( sum_d (X[b,v,d]-center[b,c,d]) * R[b,c,k,d]
                                  + center[b,c,k] + V_nodes[b,c,k] )
which factors into a vertex-independent per-node affine map:
    t[b,c,k]   = center[b,c,k] + V_nodes[b,c,k] - sum_d center[b,c,d]*R[b,c,k,d]
    out[b,v,k] = sum_d X[b,v,d] * (W @ R[..,k,d])[v]  +  (W @ t[..,k])[v]
i.e. one (V,C)@(C,48) matmul Y = W @ G, then a tiny per-vertex contraction
of Y with [X,1].  The big tensors (W: 32MB, X, out) are sharded over the
vertex dimension across the 8 cores; G (C x 48) is replicated.

Layouts: the 48 Y rows live at PSUM/SBUF partitions j = d*32 + (k*4 + b)
(d in 0..3 with d==3 the translation/ones slot; unused rows of each
32-block are zero) so every 2-input engine op sees equal base partitions,
which the ISA requires for SBUF+SBUF operand pairs.

fp32 matmul on TRN2 runs in LOW_HIGH dual-pass mode (~5x slower), so the
matmul uses the exact-enough 3-term bf16 split:
    W @ G ~= Wh@Gh + Wl@Gh + Wh@Gl     (Wh=bf16(W), Wl=bf16(W-Wh), ...)
measured end-to-end error vs the fp32 reference: ~4e-6 absmax.

The contraction dim C=160 splits into an A part (c 0..127, K=128) and a B
part (c 128..159, K=32).  The three B-part terms are packed into one K=96
matmul by stacking [WhB; WhB; WlB] against [GhB; GlB; GhB] host-side.

Device kernel per core (vertex shard Vs=6250 padded to 6272), DMAs batched
in 3072-wide macro chunks to amortize the ~0.7us per-DMA issue cost on the
sequencer queues; compute runs in 1024-wide sub-chunks:
  per macro chunk m (3072):
    DMA(SP)   wha(128,m) wla(128,m) bpk(96,m) bf16
    DMA(ACT)  xdt(128,m) f32, one DMA into the 4 partition blocks
    per 1024 sub-chunk n:
      PE    y(128,n) PSUM f32 += gh0.T@wha + gl0.T@wha + gh0.T@wla + gbk.T@bpk
            (per 512-wide half; 8 matmuls per sub-chunk)
      DVE   p (128,n) SBUF = y * xdt
      ACT   q (64,n) = copy p[64:128]          (partition shift 64 -> 0)
      POOL  a64[0:32]  = p[0:32]  + q[0:32]    (s0 + s2)
      DVE   a64[32:64] = p[32:64] + q[32:64]   (s1 + s3)
    DMA(Q7)   os (12,m) SBUF = a64[0:12]; os += a64[32:44]  (SWDGE accum)
    DMA(SP)   outT[:, m] = os
"""

import numpy as np
import ml_dtypes

import concourse.mybir as mybir
import concourse.tile as tile
from concourse import bacc
from concourse.bass_utils import run_bass_kernel_spmd

B, V, C = 4, 50000, 160
N_CORES = 8
VS = V // N_CORES            # 6250 vertices per core
VSP = 6272                   # padded vertex shard
# macro DMA chunks ramp up so compute starts as soon as the first small
# chunk lands, then amortize DMA-issue cost with bigger chunks
MACROS = [512, 1024, 2048, 2048, 640]
SUB = 1024
F32 = mybir.dt.float32
BF16 = mybir.dt.bfloat16
NPBF16 = ml_dtypes.bfloat16


def _build_bass():
    nc = bacc.Bacc()

    wha_d = nc.dram_tensor("wha", [128, VSP], BF16, kind="ExternalInput")
    wla_d = nc.dram_tensor("wla", [128, VSP], BF16, kind="ExternalInput")
    wb_d = nc.dram_tensor("wb", [96, VSP], BF16, kind="ExternalInput")
    xd_d = nc.dram_tensor("xd", [48, VSP], F32, kind="ExternalInput")
    gh0_d = nc.dram_tensor("gh0", [128, 128], BF16, kind="ExternalInput")
    gl0_d = nc.dram_tensor("gl0", [128, 128], BF16, kind="ExternalInput")
    gbk_d = nc.dram_tensor("gbk", [96, 128], BF16, kind="ExternalInput")
    outT = nc.dram_tensor("outT", [12, VSP], F32, kind="ExternalOutput")

    with tile.TileContext(nc) as tc:
        with (
            tc.tile_pool(name="gpool", bufs=1) as gpool,
            tc.tile_pool(name="wpool", bufs=2) as wpool,
            tc.tile_pool(name="xpool", bufs=2) as xpool,
            tc.tile_pool(name="ppool", bufs=3) as ppool,
            tc.tile_pool(name="qpool", bufs=3) as qpool,
            tc.tile_pool(name="apool", bufs=3) as apool,
            tc.tile_pool(name="ospool", bufs=2) as ospool,
            tc.tile_pool(name="ypool", bufs=3, space="PSUM") as ypool,
        ):
            gh0 = gpool.tile([128, 128], BF16)
            nc.sync.dma_start(out=gh0[:], in_=gh0_d[:])
            gl0 = gpool.tile([128, 128], BF16)
            nc.sync.dma_start(out=gl0[:], in_=gl0_d[:])
            gbk = gpool.tile([96, 128], BF16)
            nc.sync.dma_start(out=gbk[:], in_=gbk_d[:])

            # PE HAM warmup: ~3.5us of dummy matmuls while the first input
            # DMAs are still in flight, so real matmuls start at 2.4GHz
            # instead of the cold 1.2GHz gate. Inputs: gh0 (tiny, lands
            # early) against an uninitialized scratch tile; output PSUM is
            # never read.
            wsc = gpool.tile([128, 512], BF16)
            nc.vector.memset(wsc[:], 0.0)
            ywarm = ypool.tile([128, 512], F32, tag="ywarm", bufs=1)
            for w in range(8):
                nc.tensor.matmul(ywarm[:, :], gh0[:, 0:128], wsc[:, :],
                                 start=(w == 0), stop=(w == 7),
                                 skip_group_check=True)

            m0 = 0
            for mn in MACROS:
                msl = slice(m0, m0 + mn)
                wha = wpool.tile([128, mn], BF16, tag="wha")
                nc.sync.dma_start(out=wha[:], in_=wha_d[:, msl])
                wla = wpool.tile([128, mn], BF16, tag="wla")
                nc.sync.dma_start(out=wla[:], in_=wla_d[:, msl])
                bpk = wpool.tile([96, mn], BF16, tag="bpk")
                nc.sync.dma_start(out=bpk[:], in_=wb_d[:, msl])
                xdt = xpool.tile([128, mn], F32, tag="xdt")
                for s in range(4):
                    nc.gpsimd.dma_start(out=xdt[s * 32:s * 32 + 12, :],
                                        in_=xd_d[s * 12:(s + 1) * 12, msl])

                os_ = ospool.tile([12, mn], F32, tag="os")

                for u0 in range(0, mn, SUB):
                    n = min(SUB, mn - u0)
                    us = slice(u0, u0 + n)
                    y = ypool.tile([128, n], F32, tag="y")
                    for h in range(0, n, 512):
                        hs = slice(u0 + h, u0 + min(h + 512, n))
                        ys = slice(h, min(h + 512, n))
                        nc.tensor.matmul(y[:, ys], gh0[:], wha[:, hs],
                                         start=True, stop=False)
                        nc.tensor.matmul(y[:, ys], gl0[:], wha[:, hs],
                                         start=False, stop=False)
                        nc.tensor.matmul(y[:, ys], gh0[:], wla[:, hs],
                                         start=False, stop=False)
                        nc.tensor.matmul(y[:, ys], gbk[:], bpk[:, hs],
                                         start=False, stop=True)

                    p = ppool.tile([128, n], F32, tag="p")
                    nc.vector.tensor_mul(out=p[:], in0=y[:], in1=xdt[:, us])
                    q = qpool.tile([64, n], F32, tag="q")
                    nc.scalar.copy(out=q[:], in_=p[64:128, :])
                    a = apool.tile([64, n], F32, tag="a")
                    nc.vector.tensor_add(out=a[:], in0=p[0:64, :], in1=q[:])
                    r = qpool.tile([32, n], F32, tag="r")
                    nc.scalar.copy(out=r[:], in_=a[32:64, :])
                    nc.vector.tensor_add(out=os_[:, us], in0=a[0:12, :],
                                         in1=r[0:12, :])

                nc.sync.dma_start(out=outT[:, msl], in_=os_[:])
                m0 += mn
    nc.finalize()
    return nc


_NC_CACHE = None


def _get_nc():
    global _NC_CACHE
    if _NC_CACHE is None:
        _NC_CACHE = _build_bass()
    return _NC_CACHE


def _host_prep(X, V_nodes, rot6d_nodes, W_nodes, idx_nn_to_nodes):
    """Small per-node math (B*C=640 rows) + shard/layout of the big tensors."""
    X = np.asarray(X, np.float32)
    Vn = np.asarray(V_nodes, np.float32)
    d6 = np.asarray(rot6d_nodes, np.float32)
    W = np.asarray(W_nodes, np.float32)
    idx = np.asarray(idx_nn_to_nodes).astype(np.int64)

    a1, a2 = d6[..., :3], d6[..., 3:]
    eps = np.float32(1e-8)
    n1 = np.sqrt(np.sum(a1 * a1, -1, keepdims=True, dtype=np.float32))
    b1 = a1 / np.maximum(n1, eps)
    dot = np.sum(b1 * a2, -1, keepdims=True, dtype=np.float32)
    a2p = a2 - dot * b1
    n2 = np.sqrt(np.sum(a2p * a2p, -1, keepdims=True, dtype=np.float32))
    b2 = a2p / np.maximum(n2, eps)
    b3 = np.cross(b1, b2)
    R = np.stack([b1, b2, b3], axis=-2).astype(np.float32)  # (B,C,3,3) [b,c,k,d]

    center = X[:, idx, :]                                   # (B,C,3)
    t = (center + Vn - np.einsum('bcd,bckd->bck', center, R)).astype(np.float32)

    # G columns at j = d*32 + k*4 + b; cols 12..31 of each block zero
    G = np.zeros((C, 128), np.float32)
    for d in range(4):
        for k in range(3):
            for b in range(B):
                j = d * 32 + k * 4 + b
                G[:, j] = R[b, :, k, d] if d < 3 else t[b, :, k]

    Gh = G.astype(NPBF16)
    Gl = (G - Gh.astype(np.float32)).astype(NPBF16)
    gh0 = np.ascontiguousarray(Gh[0:128])
    gl0 = np.ascontiguousarray(Gl[0:128])
    gbk = np.ascontiguousarray(
        np.concatenate([Gh[128:160], Gl[128:160], Gh[128:160]], axis=0))

    Wh = W.astype(NPBF16)
    Wl = (W - Wh.astype(np.float32)).astype(NPBF16)

    in_maps = []
    for i in range(N_CORES):
        vsl = slice(i * VS, (i + 1) * VS)
        wht = np.zeros((160, VSP), NPBF16)
        wht[:, :VS] = Wh[vsl].T
        wlt = np.zeros((160, VSP), NPBF16)
        wlt[:, :VS] = Wl[vsl].T
        wha = np.ascontiguousarray(wht[0:128])
        wla = np.ascontiguousarray(wlt[0:128])
        wb = np.ascontiguousarray(
            np.concatenate([wht[128:160], wht[128:160], wlt[128:160]], axis=0))
        # xd rows s*12 + r with r = k*4+b: X[b,:,d] for s=d<3, ones for s=3
        xd = np.zeros((48, VSP), np.float32)
        for d in range(4):
            for k in range(3):
                for b in range(B):
                    r = d * 12 + k * 4 + b
                    xd[r, :VS] = X[b, vsl, d] if d < 3 else 1.0
        in_maps.append({"wha": wha, "wla": wla, "wb": wb, "xd": xd,
                        "gh0": gh0, "gl0": gl0, "gbk": gbk})
    return in_maps


def _gather(results):
    out = np.empty((B, V, 3), np.float32)
    for i, res in enumerate(results):
        oT = res["outT"]
        vsl = slice(i * VS, (i + 1) * VS)
        for k in range(3):
            for b in range(B):
                out[b, vsl, k] = oT[k * 4 + b, :VS]
    return out


def kernel(X, V_nodes, rot6d_nodes, W_nodes, idx_nn_to_nodes, **run_kwargs):
    in_maps = _host_prep(X, V_nodes, rot6d_nodes, W_nodes, idx_nn_to_nodes)
    res = run_bass_kernel_spmd(_get_nc(), in_maps,
                               core_ids=list(range(N_CORES)), **run_kwargs)
    out = _gather(res.results)
    kernel.last_run = res
    return out
